# revision 46
# baseline (speedup 1.0000x reference)
"""GQA forward kernel for Trainium2, 8-core tensor-parallel (group-aligned).

Problem: B=2, T=2048, D=2048, 32 Q heads / 8 KV heads, head_dim 64, causal.

Sharding: core c owns KV head c and its 4 Q heads (whole GQA group), both
batches.  Output projection is row-parallel Megatron style: each core
contracts its 256 attention-output channels against its slice of Wo and the
host sums the 8 partial outputs (+ bo).

Device-side dataflow per core (matmuls in float32r unless noted, fp32 accum):
  x^T [C, T] (host-transposed)
    -> QKK proj:  lhsT = [Wq_c | Wk_c | Wk_c]  -> Q^T [256, T], K^T dup [128, T]
    -> V proj (fp16): lhsT = x^T fp16, rhs = Wv_c fp16 -> V [T, 64] natural
  attention per (batch, head-pair, q-chunk of 512), scores TRANSPOSED:
    S^T[kv, q] = matmul(lhsT=K^T tile [64,128], rhs=Q^T [64, 512])
      head pairs run on disjoint PE row groups (base partitions 0 / 64)
    expS = ACT Exp(S^T / 8)  (no max-subtraction: |scores| <= ~6)
    causal: column-sliced matmuls + one triangle mask on diagonal tiles
    AV: matmul(lhsT=V2 [kv,65] (V plus ones col), rhs=expS) accumulated over
        kv tiles -> [attn^T; den] in PSUM
    normalize: den replicated to 64 partitions via K=1 ones-matmul,
        reciprocal + multiply on DVE
  out-proj: y[t, e] = matmul(lhsT=attn^T [256, t], rhs=Wo_c [256, e])
"""

import os

import numpy as np

import concourse.mybir as mybir
import concourse.tile as tile
from concourse import bacc
from concourse import bass_utils

P = 128
B = 2
T = 2048
C = 2048
HD = 64
QH = 32
KVH = 8
G = QH // KVH  # 4
NCORES = 8
QH_LOC = QH // NCORES  # 4 q heads per core
TCH = 256  # token chunk for projection phase
QCH = 512  # q chunk for attention phase
KT = C // P  # 16 contraction tiles
f32 = mybir.dt.float32
f32r = mybir.dt.float32r
bf16 = mybir.dt.bfloat16
fp16 = mybir.dt.float16

_CACHE = {}


def _build():
    nc = bacc.Bacc("TRN2", target_bir_lowering=False, debug=False, num_devices=NCORES)

    xt = nc.dram_tensor("xt", [B, C, T], f32, kind="ExternalInput")
    xtb = nc.dram_tensor("xtb", [B, C, T], fp16, kind="ExternalInput")
    wqk = nc.dram_tensor("wqk", [C, 384], f32, kind="ExternalInput")
    wv = nc.dram_tensor("wv", [C, HD], fp16, kind="ExternalInput")
    wo = nc.dram_tensor("wo", [G * HD, C], f32, kind="ExternalInput")
    bqk = nc.dram_tensor("bqk", [P, 3], f32, kind="ExternalInput")
    bv = nc.dram_tensor("bv", [1, HD], f32, kind="ExternalInput")
    maskd = nc.dram_tensor("mask", [P, P], f32, kind="ExternalInput")
    y = nc.dram_tensor("y", [B, T, C], f32, kind="ExternalOutput")

    wqk3 = wqk.ap().rearrange("(ko p) m -> p ko m", p=P).bitcast(f32r)
    wv3 = wv.ap().rearrange("(ko p) m -> p ko m", p=P)
    wo3 = wo.ap().rearrange("(ko p) m -> p ko m", p=P).bitcast(f32r)

    with tile.TileContext(nc) as tc:
        with (
            tc.tile_pool(name="const", bufs=1) as cpool,
            tc.tile_pool(name="x", bufs=2) as xpool,
            tc.tile_pool(name="proj", bufs=1) as projpool,
            tc.tile_pool(name="attn", bufs=1) as apool,
            tc.tile_pool(name="work", bufs=5) as wpool,
            tc.tile_pool(name="work2", bufs=6) as wpool2,
            tc.tile_pool(name="psA", bufs=2, space="PSUM") as psumA,
            tc.tile_pool(name="psB", bufs=2, space="PSUM") as psumB,
            tc.tile_pool(name="psC", bufs=2, space="PSUM") as psumC,
        ):
            # ---- constants / weights (resident) ----
            # startup-critical DMA order: wqk sub0, then x chunk 0 (the first
            # 16 QKK matmuls need only these), then the rest
            wqk_sb = cpool.tile([P, KT, 384], f32r)
            nc.sync.dma_start(wqk_sb[:, :, 0:P], wqk3[:, :, 0:P])
            xb0 = xt.ap()[0].rearrange("(ko p) t -> p ko t", p=P).bitcast(f32r)
            xbb0 = xtb.ap()[0].rearrange("(ko p) t -> p ko t", p=P)
            xch0 = xpool.tile([P, KT, TCH], f32r, tag="xch", name="xch")
            nc.sync.dma_start(xch0[:, 0 : KT // 2, :], xb0[:, 0 : KT // 2, 0:TCH])
            nc.sync.dma_start(xch0[:, KT // 2 :, :], xb0[:, KT // 2 :, 0:TCH])
            for _s in range(1, 3):
                nc.sync.dma_start(
                    wqk_sb[:, :, _s * P : (_s + 1) * P], wqk3[:, :, _s * P : (_s + 1) * P]
                )
            xchb0 = xpool.tile([P, KT, TCH], fp16, tag="xchb", name="xchb")
            nc.sync.dma_start(xchb0[:, 0 : KT // 2, :], xbb0[:, 0 : KT // 2, 0:TCH])
            nc.sync.dma_start(xchb0[:, KT // 2 :, :], xbb0[:, KT // 2 :, 0:TCH])
            wv_sb = cpool.tile([P, KT, HD], fp16)
            nc.sync.dma_start(wv_sb[:], wv3)
            bqk_sb = cpool.tile([P, 3], f32)
            nc.sync.dma_start(bqk_sb[:], bqk.ap())
            bv_sb = cpool.tile([P, HD], f32)
            nc.sync.dma_start(bv_sb[:], bv.ap().to_broadcast((P, HD)))
            mask_sb = cpool.tile([P, P], f32r)
            nc.sync.dma_start(mask_sb[:], maskd.ap().bitcast(f32r))
            ones_f32 = cpool.tile([P, KT], f32)
            nc.gpsimd.memset(ones_f32[:], 1.0)
            ones_r = cpool.tile([P, HD], f32r)
            nc.vector.tensor_copy(ones_r[:], ones_f32[:, 0:1].to_broadcast((P, HD)))
            wo_sb = cpool.tile([P, 2, C], f32r)

            def emit_p3(pb, pattn, pqc):
                for ts in range(pqc * (QCH // P), (pqc + 1) * (QCH // P)):
                    for ec in range(C // QCH):
                        py = psumC.tile([P, QCH], f32, tag="pp", name="py")
                        for ks in range(2):
                            nc.tensor.matmul(
                                py[:],
                                pattn[:, ks, ts * P : (ts + 1) * P],
                                wo_sb[:, ks, ec * QCH : (ec + 1) * QCH],
                                start=(ks == 0),
                                stop=(ks == 1),
                            )
                        y_sb = wpool2.tile([P, QCH], f32, tag="ysb")
                        nc.any.tensor_copy(y_sb[:], py[:])
                        nc.sync.dma_start(
                            y.ap()[
                                pb, ts * P : (ts + 1) * P, ec * QCH : (ec + 1) * QCH
                            ],
                            y_sb[:],
                        )

            deferred_p3 = None
            for b in range(B):
                xb = xt.ap()[b].rearrange("(ko p) t -> p ko t", p=P).bitcast(f32r)
                xbb = xtb.ap()[b].rearrange("(ko p) t -> p ko t", p=P)

                # ---- P1: projections ----
                qkk_sb = projpool.tile([P, 3, T], f32r, tag="qkk")
                v2_sb = projpool.tile([P, KT, 130], f32r, tag="v2")
                nc.vector.tensor_copy(v2_sb[:, :, 64:65], ones_f32[:, :, None])
                for tch in range(T // TCH):
                    tsl = slice(tch * TCH, (tch + 1) * TCH)
                    if b == 0 and tch == 0:
                        xch, xchb = xch0, xchb0
                    else:
                        xch = xpool.tile([P, KT, TCH], f32r, tag="xch", name="xch")
                        nc.sync.dma_start(xch[:, 0 : KT // 2, :], xb[:, 0 : KT // 2, tsl])
                        nc.sync.dma_start(xch[:, KT // 2 :, :], xb[:, KT // 2 :, tsl])
                        xchb = xpool.tile([P, KT, TCH], fp16, tag="xchb", name="xchb")
                        nc.sync.dma_start(xchb[:, 0 : KT // 2, :], xbb[:, 0 : KT // 2, tsl])
                        nc.sync.dma_start(xchb[:, KT // 2 :, :], xbb[:, KT // 2 :, tsl])
                    if tch == 1 and b == 0:
                        nc.sync.dma_start(wo_sb[:], wo3)
                    if tch == 3 and deferred_p3 is not None:
                        emit_p3(*deferred_p3)
                        deferred_p3 = None
                    for sub in range(3):
                        pp_full = psumC.tile([P, QCH], f32, tag="pp", name="pp")
                        pp = pp_full[:, :TCH]
                        for k in range(KT):
                            nc.tensor.matmul(
                                pp[:],
                                wqk_sb[:, k, sub * P : (sub + 1) * P],
                                xch[:, k, :],
                                start=(k == 0),
                                stop=(k == KT - 1),
                            )
                        nc.any.tensor_tensor(
                            qkk_sb[:, sub, tsl],
                            pp[:],
                            bqk_sb[:, sub : sub + 1].to_broadcast((P, TCH)),
                            mybir.AluOpType.add,
                        )
                    for ts in range(TCH // P):
                        tidx = tch * (TCH // P) + ts
                        pv = psumC.tile([P, HD], f32, tag="pp", name="pv")
                        for k in range(KT):
                            nc.tensor.matmul(
                                pv[:],
                                xchb[:, k, ts * P : (ts + 1) * P],
                                wv_sb[:, k, :],
                                start=(k == 0),
                                stop=(k == KT - 1),
                            )
                        nc.any.tensor_tensor(
                            v2_sb[:, tidx, 0:64], pv[:], bv_sb[:], mybir.AluOpType.add
                        )
                        nc.any.tensor_tensor(
                            v2_sb[:, tidx, 65:129], pv[:], bv_sb[:], mybir.AluOpType.add
                        )

                # ---- P2 + P3 interleaved: attention then out-proj per q-chunk ----
                # Head pairs (2*sub, 2*sub+1) run QK^T on disjoint PE row
                # groups (base partitions 0 / 64); their score tiles share one
                # 2-bank PSUM tile so exp is a single wide ACT op.
                attn_sb = apool.tile([P, 2, T], f32r, tag="attn")
                for qc in range(T // QCH):
                    q0 = qc * QCH
                    nfull = q0 // P
                    ntiles = nfull + QCH // P
                    for sub in range(2):
                        qT0 = qkk_sb[0:64, sub, q0 : q0 + QCH]
                        qT1 = qkk_sb[64:128, sub, q0 : q0 + QCH]
                        pav0 = psumB.tile([P, QCH], f32, tag="pav", name="pav0")
                        pav1 = psumB.tile([P, QCH], f32, tag="pav", name="pav1")
                        for i in range(ntiles):
                            if i < nfull:
                                nsl = slice(0, QCH)
                            else:
                                nsl = slice((i - nfull) * P, QCH)
                            ksl = slice(i * P, (i + 1) * P)
                            ps_s = psumA.tile([P, 2, QCH], f32, tag="ps", name="ps_s")
                            # concurrent pair: disjoint PE row groups 0-63 / 64-127
                            nc.tensor.matmul(
                                ps_s[:, 0, nsl],
                                qkk_sb[0:64, 2, ksl],
                                qT0[:, nsl],
                                start=True,
                                stop=True,
                            )
                            nc.tensor.matmul(
                                ps_s[:, 1, nsl],
                                qkk_sb[64:128, 2, ksl],
                                qT1[:, nsl],
                                start=True,
                                stop=True,
                            )
                            expS = wpool.tile([P, 2, QCH], f32r, tag="expS")
                            nc.scalar.activation(
                                expS[:, :, nsl],
                                ps_s[:, :, nsl],
                                mybir.ActivationFunctionType.Exp,
                                scale=0.125,
                            )
                            if i >= nfull:
                                j = i - nfull
                                nc.any.tensor_tensor(
                                    expS[:, :, j * P : (j + 1) * P],
                                    expS[:, :, j * P : (j + 1) * P],
                                    mask_sb[:, None, :].to_broadcast((P, 2, P)),
                                    mybir.AluOpType.mult,
                                )
                            for half, pav in ((0, pav0), (1, pav1)):
                                nc.tensor.matmul(
                                    pav[0:65, nsl],
                                    v2_sb[:, i, 0:65],
                                    expS[:, half, nsl],
                                    start=(i == 0),
                                    stop=(i == ntiles - 1),
                                    skip_group_check=True,
                                )
                        for half, pav in ((0, pav0), (1, pav1)):
                            den_sb = wpool2.tile([P, QCH], f32r, tag="den")
                            nc.any.tensor_copy(den_sb[64:65, :], pav[64:65, :])
                            ps_den = psumA.tile([64, QCH], f32, tag="ps", name="psd")
                            nc.tensor.matmul(
                                ps_den[:],
                                ones_r[64:65, 0:64],
                                den_sb[64:65, :],
                                start=True,
                                stop=True,
                            )
                            rec = wpool2.tile([64, QCH], f32, tag="rec")
                            nc.vector.reciprocal(rec[:], ps_den[:])
                            if half == 0:
                                nc.any.tensor_tensor(
                                    attn_sb[0:64, sub, q0 : q0 + QCH],
                                    pav[0:64, :],
                                    rec[:],
                                    mybir.AluOpType.mult,
                                )
                            else:
                                alo = wpool2.tile([64, QCH], f32r, tag="alo")
                                nc.any.tensor_tensor(
                                    alo[:], pav[0:64, :], rec[:], mybir.AluOpType.mult
                                )
                                nc.sync.dma_start(
                                    attn_sb[64:128, sub, q0 : q0 + QCH], alo[:]
                                )

                    # out-proj for the finished token range; the last q-chunk is
                    # deferred into the next batch's P1 (fills PE during DMA waits)
                    if qc < T // QCH - 1 or b == B - 1:
                        emit_p3(b, attn_sb, qc)
                    else:
                        deferred_p3 = (b, attn_sb, qc)

            if deferred_p3 is not None:
                emit_p3(*deferred_p3)

    nc.compile()
    return nc


def _prep_inputs(x, Wq, bq, Wk, bk, Wv, bv, Wo, bo):
    x = np.ascontiguousarray(np.asarray(x, dtype=np.float32))
    xt = np.ascontiguousarray(x.transpose(0, 2, 1))
    xtb = xt.astype(np.float16)
    Wq = np.asarray(Wq, dtype=np.float32)
    Wk = np.asarray(Wk, dtype=np.float32)
    Wv = np.asarray(Wv, dtype=np.float32)
    Wo = np.asarray(Wo, dtype=np.float32)
    bq = np.asarray(bq, dtype=np.float32)
    bk = np.asarray(bk, dtype=np.float32)
    bv = np.asarray(bv, dtype=np.float32)

    # mask[kj, qi] = 1 iff kj <= qi  (upper triangular incl. diag)
    mask = np.triu(np.ones((P, P), dtype=np.float32)).copy()
    in_maps = []
    for c in range(NCORES):
        qs = slice(c * G * HD, (c + 1) * G * HD)
        ks = slice(c * HD, (c + 1) * HD)
        wqk_c = np.concatenate([Wq[:, qs], Wk[:, ks], Wk[:, ks]], axis=1)
        bq_c = bq[qs]
        bqk_c = np.stack(
            [bq_c[0:128], bq_c[128:256], np.concatenate([bk[ks], bk[ks]])], axis=1
        )
        in_maps.append(
            {
                "xt": xt,
                "xtb": xtb,
                "wqk": np.ascontiguousarray(wqk_c),
                "wv": np.ascontiguousarray(Wv[:, ks]).astype(np.float16),
                "wo": np.ascontiguousarray(Wo[qs, :]),
                "bqk": np.ascontiguousarray(bqk_c),
                "bv": np.ascontiguousarray(bv[None, ks]),
                "mask": mask,
            }
        )
    return in_maps


def kernel(x, Wq, bq, Wk, bk, Wv, bv, Wo, bo, _trace=False):
    # NTFF tracing is unavailable through this axon client; make sure a
    # stray BASS_TRACE=1 in the environment cannot divert the run path.
    if not _trace:
        os.environ["BASS_NEVER_TRACE"] = "1"
    if "nc" not in _CACHE:
        _CACHE["nc"] = _build()
    nc = _CACHE["nc"]
    in_maps = _prep_inputs(x, Wq, bq, Wk, bk, Wv, bv, Wo, bo)
    res = bass_utils.run_bass_kernel_spmd(
        nc, in_maps, core_ids=list(range(NCORES)), trace=_trace
    )
    bo = np.asarray(bo, dtype=np.float32)
    y = np.zeros((B, T, C), dtype=np.float32)
    for c in range(NCORES):
        y += res.results[c]["y"]
    y += bo
    if _trace:
        return y, res
    return y
